# revision 12
# baseline (speedup 1.0000x reference)
"""Trainium2 Bass kernel for BatchWiseTripletDistanceLoss.

Math: loss = mean_t relu(cos_d(s[a_t], s[p_t]) - cos_d(s[a_t], s[n_t]) + margin)
with cos_d(x, y) = 1 - <x,y>/max(|x||y|, eps).

Cosine distances depend only on (row, row) pairs of the 512x256 sample
matrix, so the kernel computes the 512x512 cosine-SIMILARITY matrix
sim = R S S^T R (R = diag(1/|s_i|)) on-device via TensorE and evaluates
    relu(sim[a,p] - sim[a,n] + margin)          ("1-" cancels in the diff)
on a dense [row, col] grid: each triplet is scattered to grid cell
(a_t, n_t) carrying p_t+1 (gpsimd local_scatter = true per-partition
scatter).  The distinct positives of each row form a small palette
(~13 entries here); palette VALUES are extracted from the sim row by a
second local_scatter, and a short loop over palette slots evaluates
masked relu terms, so no per-triplet gather is ever needed.

Sharding: 8 cores split the grid into (row mod 4) x (column half)
quadrants of [128, 256].  The host only transposes/permutes/buckets/pads
the given arrays (layout + palette metadata, no float math) and sums the
8 partial scalars at the end.
"""
import sys

sys.path.insert(0, "/opt/trn_rl_repo")

from contextlib import ExitStack

import numpy as np

import concourse.bacc as bacc
import concourse.bass as bass
import concourse.tile as tile
from concourse import mybir
from concourse.bass_utils import run_bass_kernel_spmd

DT = mybir.dt
OP = mybir.AluOpType
ACTF = mybir.ActivationFunctionType

N = 512
D = 256
MARGIN = 0.15
NCORES = 8
LCOL = 256  # columns per core (half)
NROW = 128  # rows per core (stride-4 residue class)


def _build_program(s_pad: int):
    """Build + compile the SPMD program (identical for all 8 cores)."""
    nc = bacc.Bacc(
        "TRN2", target_bir_lowering=False, debug=False, num_devices=NCORES
    )
    f32, i32, i16, f16 = DT.float32, DT.int32, DT.int16, DT.float16

    d_stp = nc.dram_tensor("stp", [D, N], f32, kind="ExternalInput").ap()
    d_srt = nc.dram_tensor("srt", [D, NROW], f32, kind="ExternalInput").ap()
    d_sidx = nc.dram_tensor("sidx16", [NROW, N], i16, kind="ExternalInput").ap()
    d_pal = nc.dram_tensor("palidx1f", [NROW, s_pad], f32, kind="ExternalInput").ap()
    d_nbuk = nc.dram_tensor("nbuk16", [NROW, LCOL], i16, kind="ExternalInput").ap()
    d_pbuk = nc.dram_tensor("pbuk16", [NROW, LCOL], f16, kind="ExternalInput").ap()
    d_out = nc.dram_tensor("out", [1, 1], f32, kind="ExternalOutput").ap()

    with tile.TileContext(nc) as tc, ExitStack() as ctx:
        cpool = ctx.enter_context(tc.tile_pool(name="const", bufs=1))
        wpool = ctx.enter_context(tc.tile_pool(name="work", bufs=2))
        mpool = ctx.enter_context(tc.tile_pool(name="mainloop", bufs=4))
        ppool = ctx.enter_context(tc.tile_pool(name="psum", bufs=2, space="PSUM"))
        pfin = ctx.enter_context(tc.tile_pool(name="psumfin", bufs=1, space="PSUM"))
        pbig = ctx.enter_context(tc.tile_pool(name="psumbig", bufs=1, space="PSUM"))

        # ---- load inputs -------------------------------------------------
        st = []
        for k in range(2):
            t = cpool.tile([128, N], f32, tag=f"st{k}", name=f"st{k}")
            nc.sync.dma_start(t[:], d_stp[128 * k : 128 * (k + 1), :])
            st.append(t)
        sr = []
        for k in range(2):
            t = cpool.tile([128, NROW], f32, tag=f"sr{k}", name=f"sr{k}")
            nc.sync.dma_start(t[:], d_srt[128 * k : 128 * (k + 1), :])
            sr.append(t)
        nbuk16 = cpool.tile([NROW, LCOL], i16)
        nc.sync.dma_start(nbuk16[:], d_nbuk)
        pbuk16 = cpool.tile([NROW, LCOL], f16)
        nc.sync.dma_start(pbuk16[:], d_pbuk)
        sidx16 = cpool.tile([NROW, N], i16)
        nc.sync.dma_start(sidx16[:], d_sidx)
        palidx1f = cpool.tile([NROW, s_pad], f32)
        nc.sync.dma_start(palidx1f[:], d_pal)

        ones_col = cpool.tile([128, 1], f32)
        nc.vector.memset(ones_col[:], 1.0)
        ones_row1 = cpool.tile([1, 128], f32)
        nc.vector.memset(ones_row1[:], 1.0)

        # ---- preload ACT sqrt table during the DMA phase ----------------
        dumin = cpool.tile([1, 1], f32)
        nc.vector.memset(dumin[:], 4.0)
        dumout = cpool.tile([1, 1], f32)
        nc.scalar.sqrt(dumout[:], dumin[:])

        # ---- pidxg scatter (independent of samples) ---------------------
        pidxg = wpool.tile([NROW, LCOL], f16, tag="pidxg")
        nc.gpsimd.local_scatter(
            pidxg[:], pbuk16[:], nbuk16[:],
            channels=128, num_elems=LCOL, num_idxs=LCOL,
        )

        # ---- raw similarity matmul --------------------------------------
        simp = pbig.tile([128, N], f32, tag="simp")
        for k in range(2):
            nc.tensor.matmul(simp[:], sr[k][:], st[k][:], start=(k == 0), stop=(k == 1))

        # ---- norms ------------------------------------------------------
        sq = wpool.tile([128, N], f32, tag="sq")
        n2p = pbig.tile([1, N], f32, tag="n2row")
        for k in range(2):
            nc.vector.tensor_tensor(sq[:], st[k][:], st[k][:], OP.mult)
            nc.tensor.matmul(n2p[:], ones_col[:], sq[:], start=(k == 0), stop=(k == 1))
        sqr = wpool.tile([128, NROW], f32, tag="sqr")
        n2rp = ppool.tile([128, 1], f32, tag="n2rp")
        for k in range(2):
            nc.vector.tensor_tensor(sqr[:], sr[k][:], sr[k][:], OP.mult)
            nc.tensor.matmul(n2rp[:], sqr[:], ones_col[:], start=(k == 0), stop=(k == 1))

        nrow = wpool.tile([1, N], f32, tag="nrow")
        nc.scalar.sqrt(nrow[:], n2p[:])
        nrr = wpool.tile([128, 1], f32, tag="nrr")
        nc.scalar.sqrt(nrr[:], n2rp[:])
        # preload relu table right after the sqrts (hidden off critical path)
        durelu = cpool.tile([1, 1], f32)
        nc.scalar.activation(durelu[:], dumout[:], ACTF.Relu)

        rrow = wpool.tile([1, N], f32, tag="rrow")
        rscr = wpool.tile([1, N], f32, tag="rscr")
        nc.vector.reciprocal_approx_accurate(rrow[:], nrow[:], rscr[:])
        rr = cpool.tile([128, 1], f32)
        rscr2 = wpool.tile([128, 1], f32, tag="rscr2")
        nc.vector.reciprocal_approx_accurate(rr[:], nrr[:], rscr2[:])

        # ---- combine: simrow = (raw . rr) x RB --------------------------
        rbp = pbig.tile([128, N], f32, tag="rb")
        nc.tensor.matmul(rbp[:], ones_row1[:], rrow[:], start=True, stop=True)
        t0 = wpool.tile([128, N], f32, tag="t0")
        nc.scalar.activation(t0[:], simp[:], ACTF.Copy, scale=rr[:])
        simrow = cpool.tile([128, N], f32)
        nc.vector.tensor_tensor(simrow[:], t0[:], rbp[:], OP.mult)
        sim16 = cpool.tile([128, N], f16)
        nc.scalar.copy(sim16[:], simrow[:])
        grid16 = cpool.tile([128, LCOL], f16)
        nc.scalar.copy(grid16[:], simrow[:, 0:LCOL])

        # ---- palette values + margin bias -------------------------------
        palv16 = wpool.tile([128, s_pad], f16, tag="palv16")
        nc.gpsimd.local_scatter(
            palv16[:], sim16[:], sidx16[:],
            channels=128, num_elems=s_pad, num_idxs=N,
        )
        mb = wpool.tile([128, s_pad], f32, tag="mb")
        nc.vector.tensor_scalar(mb[:], palv16[:], -1.0, MARGIN, OP.mult, OP.add)

        # ---- main palette loop ------------------------------------------
        mskdall = wpool.tile([128, s_pad * LCOL], f16, tag="mskdall")
        for s in range(s_pad):
            ms = mpool.tile([128, LCOL], f16, tag="ms", name=f"ms{s}")
            nc.gpsimd.tensor_scalar(
                ms[:], pidxg[:], palidx1f[:, s : s + 1], None, OP.is_equal
            )
            nc.vector.scalar_tensor_tensor(
                mskdall[:, s * LCOL : (s + 1) * LCOL],
                grid16[:], mb[:, s : s + 1], ms[:], OP.add, OP.mult,
            )
        rl = wpool.tile([128, s_pad * LCOL], f16, tag="rl")
        accr = wpool.tile([128, 1], f32, tag="accr")
        nc.scalar.activation(rl[:], mskdall[:], ACTF.Relu, accum_out=accr[:])

        # ---- total: sum across partitions via ones matmul ---------------
        fin = pfin.tile([1, 1], f32, tag="fin")
        nc.tensor.matmul(fin[:], accr[:], ones_col[:], start=True, stop=True)
        outs = wpool.tile([1, 1], f32, tag="outs")
        nc.scalar.copy(outs[:], fin[:])
        nc.sync.dma_start(d_out, outs[:])

    nc.compile()
    return nc


_PROGRAM_CACHE = {}


def _get_program(s_pad):
    if s_pad not in _PROGRAM_CACHE:
        _PROGRAM_CACHE[s_pad] = _build_program(s_pad)
    return _PROGRAM_CACHE[s_pad]


def _shard_inputs(samples, targets, a, p, n, s_pad):
    """Per-core layout: transpose/permute samples, bucket triplets, build
    palette metadata (distinct positives per row)."""
    in_maps = []
    for core in range(NCORES):
        R, H = core >> 1, core & 1
        rows = np.arange(NROW, dtype=np.int64) * 4 + R
        perm = np.concatenate(
            [np.arange(256 * H, 256 * H + 256), np.arange(256 * (1 - H), 256 * (2 - H))]
        )
        sel = ((a & 3) == R) & ((n >> 8) == H)
        asel, psel, nsel = a[sel], p[sel], n[sel]
        q = asel >> 2
        order = np.argsort(q, kind="stable")
        qs = q[order]
        counts = np.bincount(qs, minlength=NROW)
        if counts.max() > LCOL:
            raise ValueError("bucket overflow")
        starts = np.zeros(NROW, dtype=np.int64)
        starts[1:] = np.cumsum(counts)[:-1]
        slot = np.arange(len(qs)) - starts[qs]
        nbuk = np.full((NROW, LCOL), -1, dtype=np.int16)
        pbuk = np.zeros((NROW, LCOL), dtype=np.float16)
        nbuk[qs, slot] = (nsel[order] & 255).astype(np.int16)
        pbuk[qs, slot] = (psel[order] + 1).astype(np.float16)

        # palettes: distinct positives per row; local col of raw id v:
        # (v & 255) + 256 * (v >> 8 != H)
        sidx = np.full((NROW, N), -1, dtype=np.int16)
        palidx1 = np.full((NROW, s_pad), -1.0, dtype=np.float32)  # -1 matches nothing
        ar = a[(a & 3) == R]
        pr = p[(a & 3) == R]
        rr_ = ar >> 2
        for qq in range(NROW):
            vals = np.unique(pr[rr_ == qq])
            if len(vals) > s_pad:
                raise ValueError("palette overflow")
            if len(vals) == 0:
                continue
            lcols = (vals & 255) + 256 * ((vals >> 8) != H)
            sidx[qq, lcols] = np.arange(len(vals), dtype=np.int16)
            palidx1[qq, : len(vals)] = vals + 1.0
        in_maps.append(
            {
                "stp": np.ascontiguousarray(samples[perm].T),
                "srt": np.ascontiguousarray(samples[rows].T),
                "sidx16": sidx,
                "palidx1f": palidx1,
                "nbuk16": nbuk,
                "pbuk16": pbuk,
            }
        )
    return in_maps


def kernel(samples, targets, anchor_idx, pos_idx, neg_idx, _want_trace=False):
    samples = np.asarray(samples, dtype=np.float32)
    targets = np.asarray(targets).astype(np.int32)
    a = np.asarray(anchor_idx).astype(np.int64)
    p = np.asarray(pos_idx).astype(np.int64)
    n = np.asarray(neg_idx).astype(np.int64)
    T = a.shape[0]
    assert samples.shape == (N, D)

    ok = (
        np.all((a >= 0) & (a < N) & (p >= 0) & (p < N) & (n >= 0) & (n < N))
        and len(np.unique(a * N + n)) == T
    )
    if not ok:
        raise NotImplementedError("inputs violate mined-triplet structure")

    ap_pairs = np.unique(a * N + p)
    npal = np.bincount(ap_pairs // N, minlength=N)
    s_max = int(npal.max())
    s_pad = max(2, s_max + (s_max & 1))
    if s_pad > 32:
        raise NotImplementedError("palette too large for this kernel")

    nc = _get_program(s_pad)
    in_maps = _shard_inputs(samples, targets, a, p, n, s_pad)
    res = run_bass_kernel_spmd(nc, in_maps, list(range(NCORES)), trace=_want_trace)
    total = sum(float(res.results[c]["out"][0, 0]) for c in range(NCORES))
    loss = np.float32(total / T)
    if _want_trace:
        return loss, res
    return loss


# revision 13
# speedup vs baseline: 1.9442x; 1.9442x over previous
"""Trainium2 Bass kernel for BatchWiseTripletDistanceLoss.

Math: loss = mean_t relu(cos_d(s[a_t], s[p_t]) - cos_d(s[a_t], s[n_t]) + margin)
with cos_d(x, y) = 1 - <x,y>/max(|x||y|, eps).

Cosine distances depend only on (row, row) pairs of the 512x256 sample
matrix, so the kernel computes the 512x512 cosine-SIMILARITY matrix
sim = R S S^T R (R = diag(1/|s_i|)) on-device via TensorE and evaluates
    relu(sim[a,p] - sim[a,n] + margin)          ("1-" cancels in the diff)
on a dense [row, col] grid: each triplet is scattered to grid cell
(a_t, n_t) carrying p_t+1 (gpsimd local_scatter = true per-partition
scatter).  The distinct positives of each row form a small palette
(~13 entries here); palette VALUES are extracted from the sim row by a
second local_scatter, and a short loop over palette slots evaluates
masked relu terms, so no per-triplet gather is ever needed.

Sharding: 8 cores split the grid into (row mod 4) x (column half)
quadrants of [128, 256].  The host only transposes/permutes/buckets/pads
the given arrays (layout + palette metadata, no float math) and sums the
8 partial scalars at the end.
"""
import sys

sys.path.insert(0, "/opt/trn_rl_repo")

from contextlib import ExitStack

import numpy as np
import ml_dtypes

ml_bf16 = ml_dtypes.bfloat16

import concourse.bacc as bacc
import concourse.bass as bass
import concourse.tile as tile
from concourse import mybir
from concourse.bass_utils import run_bass_kernel_spmd

DT = mybir.dt
OP = mybir.AluOpType
ACTF = mybir.ActivationFunctionType

N = 512
D = 256
MARGIN = 0.15
NCORES = 8
LCOL = 256  # columns per core (half)
NROW = 128  # rows per core (stride-4 residue class)


def _build_program(s_pad: int):
    """Build + compile the SPMD program (identical for all 8 cores)."""
    nc = bacc.Bacc(
        "TRN2", target_bir_lowering=False, debug=False, num_devices=NCORES
    )
    f32, i32, i16, f16 = DT.float32, DT.int32, DT.int16, DT.float16

    d_stp = nc.dram_tensor("stp", [D, N], f32, kind="ExternalInput").ap()
    d_srt = nc.dram_tensor("srt", [D, NROW], f32, kind="ExternalInput").ap()
    d_sidx = nc.dram_tensor("sidx16", [NROW, N], i16, kind="ExternalInput").ap()
    d_pal = nc.dram_tensor("palidx1f", [NROW, s_pad], f32, kind="ExternalInput").ap()
    d_nbuk = nc.dram_tensor("nbuk16", [NROW, LCOL], i16, kind="ExternalInput").ap()
    d_pbuk = nc.dram_tensor("pbuk16", [NROW, LCOL], DT.bfloat16, kind="ExternalInput").ap()
    d_ranks = nc.dram_tensor("ranks1", [NROW, s_pad], DT.bfloat16, kind="ExternalInput").ap()
    d_out = nc.dram_tensor("out", [1, 1], f32, kind="ExternalOutput").ap()

    with tile.TileContext(nc) as tc, ExitStack() as ctx:
        cpool = ctx.enter_context(tc.tile_pool(name="const", bufs=1))
        wpool = ctx.enter_context(tc.tile_pool(name="work", bufs=2))
        mpool = ctx.enter_context(tc.tile_pool(name="mainloop", bufs=4))
        ppool = ctx.enter_context(tc.tile_pool(name="psum", bufs=2, space="PSUM"))
        pfin = ctx.enter_context(tc.tile_pool(name="psumfin", bufs=1, space="PSUM"))
        pbig = ctx.enter_context(tc.tile_pool(name="psumbig", bufs=1, space="PSUM"))

        # ---- load inputs -------------------------------------------------
        st = []
        for k in range(2):
            t = cpool.tile([128, N], f32, tag=f"st{k}", name=f"st{k}")
            nc.sync.dma_start(t[:], d_stp[128 * k : 128 * (k + 1), :])
            st.append(t)
        sr = []
        for k in range(2):
            t = cpool.tile([128, NROW], f32, tag=f"sr{k}", name=f"sr{k}")
            nc.sync.dma_start(t[:], d_srt[128 * k : 128 * (k + 1), :])
            sr.append(t)
        nbuk16 = cpool.tile([NROW, LCOL], i16)
        nc.sync.dma_start(nbuk16[:], d_nbuk)
        pbuk16 = cpool.tile([NROW, LCOL], DT.bfloat16)
        nc.sync.dma_start(pbuk16[:], d_pbuk)
        sidx16 = cpool.tile([NROW, N], i16)
        nc.sync.dma_start(sidx16[:], d_sidx)
        palidx1f = cpool.tile([NROW, s_pad], f32)
        nc.sync.dma_start(palidx1f[:], d_pal)
        ranks1 = cpool.tile([NROW, s_pad], DT.bfloat16)
        nc.sync.dma_start(ranks1[:], d_ranks)

        ones_col = cpool.tile([128, 1], f32)
        nc.vector.memset(ones_col[:], 1.0)
        ones_row1 = cpool.tile([1, 128], f32)
        nc.vector.memset(ones_row1[:], 1.0)

        # ---- preload ACT sqrt table during the DMA phase ----------------
        dumin = cpool.tile([1, 1], f32)
        nc.vector.memset(dumin[:], 4.0)
        dumout = cpool.tile([1, 1], f32)
        nc.scalar.sqrt(dumout[:], dumin[:])

        # ---- pidxg scatter (independent of samples) ---------------------
        pidxg = wpool.tile([NROW, LCOL], DT.bfloat16, tag="pidxg")
        nc.gpsimd.local_scatter(
            pidxg[:], pbuk16[:], nbuk16[:],
            channels=128, num_elems=LCOL, num_idxs=LCOL,
        )

        # ---- raw similarity matmul --------------------------------------
        simp = pbig.tile([128, N], f32, tag="simp")
        for k in range(2):
            nc.tensor.matmul(simp[:], sr[k][:], st[k][:], start=(k == 0), stop=(k == 1))

        # ---- norms ------------------------------------------------------
        sq = wpool.tile([128, N], f32, tag="sq")
        n2p = pbig.tile([1, N], f32, tag="n2row")
        for k in range(2):
            nc.scalar.square(sq[:], st[k][:])
            nc.tensor.matmul(n2p[:], ones_col[:], sq[:], start=(k == 0), stop=(k == 1))
        sqr = wpool.tile([128, NROW], f32, tag="sqr")
        n2rp = ppool.tile([128, 1], f32, tag="n2rp")
        for k in range(2):
            nc.scalar.square(sqr[:], sr[k][:])
            nc.tensor.matmul(n2rp[:], sqr[:], ones_col[:], start=(k == 0), stop=(k == 1))

        nrow = wpool.tile([1, N], f32, tag="nrow")
        nc.scalar.sqrt(nrow[:], n2p[:])
        nrr = wpool.tile([128, 1], f32, tag="nrr")
        nc.scalar.sqrt(nrr[:], n2rp[:])
        # preload relu table right after the sqrts (hidden off critical path)
        durelu = cpool.tile([1, 1], f32)
        nc.scalar.activation(durelu[:], dumout[:], ACTF.Relu)

        rrow = wpool.tile([1, N], f32, tag="rrow")
        rscr = wpool.tile([1, N], f32, tag="rscr")
        nc.vector.reciprocal_approx_accurate(rrow[:], nrow[:], rscr[:])
        rr = cpool.tile([128, 1], f32)
        rscr2 = wpool.tile([128, 1], f32, tag="rscr2")
        nc.vector.reciprocal_approx_accurate(rr[:], nrr[:], rscr2[:])

        # ---- combine: simrow = (raw . rr) x RB --------------------------
        rbp = pbig.tile([128, N], f32, tag="rb")
        nc.tensor.matmul(rbp[:], ones_row1[:], rrow[:], start=True, stop=True)
        t0 = wpool.tile([128, N], f32, tag="t0")
        nc.scalar.activation(t0[:], simp[:], ACTF.Copy, scale=rr[:])
        simrow = cpool.tile([128, N], f32)
        nc.vector.tensor_tensor(simrow[:], t0[:], rbp[:], OP.mult)
        sim16 = cpool.tile([128, N], f16)
        nc.scalar.copy(sim16[:], simrow[:])

        # ---- palette values + margin bias -------------------------------
        palv16 = wpool.tile([128, s_pad], f16, tag="palv16")
        nc.gpsimd.local_scatter(
            palv16[:], sim16[:], sidx16[:],
            channels=128, num_elems=s_pad, num_idxs=N,
        )
        palvf = wpool.tile([128, s_pad], f32, tag="palvf")
        nc.scalar.copy(palvf[:], palv16[:])
        mb = wpool.tile([128, s_pad], f32, tag="mb")
        nc.vector.tensor_scalar(mb[:], palvf[:], -1.0, MARGIN, OP.mult, OP.add)

        # ---- main palette loop (batched via stride-0 broadcast APs) -----
        msall = wpool.tile([128, s_pad * LCOL], DT.bfloat16, tag="msall")
        msall_v = msall[:].rearrange("p (s j) -> p s j", s=s_pad)
        nc.vector.tensor_tensor(
            msall_v,
            pidxg[:].unsqueeze(1).to_broadcast((NROW, s_pad, LCOL)),
            ranks1[:].unsqueeze(2).to_broadcast((NROW, s_pad, LCOL)),
            OP.is_equal,
        )
        t1all = wpool.tile([128, s_pad * LCOL], f32, tag="t1all")
        nc.vector.tensor_tensor(
            t1all[:].rearrange("p (s j) -> p s j", s=s_pad),
            simrow[:, 0:LCOL].unsqueeze(1).to_broadcast((NROW, s_pad, LCOL)),
            mb[:].unsqueeze(2).to_broadcast((NROW, s_pad, LCOL)),
            OP.add,
        )
        mskdall = wpool.tile([128, s_pad * LCOL], f32, tag="mskdall")
        nc.vector.tensor_tensor(mskdall[:], msall[:], t1all[:], OP.mult)
        rl = wpool.tile([128, s_pad * LCOL], f32, tag="rl")
        accr = wpool.tile([128, 1], f32, tag="accr")
        nc.scalar.activation(rl[:], mskdall[:], ACTF.Relu, accum_out=accr[:])

        # ---- total: sum across partitions via ones matmul ---------------
        fin = pfin.tile([1, 1], f32, tag="fin")
        nc.tensor.matmul(fin[:], accr[:], ones_col[:], start=True, stop=True)
        outs = wpool.tile([1, 1], f32, tag="outs")
        nc.scalar.copy(outs[:], fin[:])
        nc.sync.dma_start(d_out, outs[:])

    nc.compile()
    return nc


_PROGRAM_CACHE = {}


def _get_program(s_pad):
    if s_pad not in _PROGRAM_CACHE:
        _PROGRAM_CACHE[s_pad] = _build_program(s_pad)
    return _PROGRAM_CACHE[s_pad]


def _shard_inputs(samples, targets, a, p, n, s_pad):
    """Per-core layout: transpose/permute samples, bucket triplets, build
    palette metadata (distinct positives per row)."""
    in_maps = []
    for core in range(NCORES):
        R, H = core >> 1, core & 1
        rows = np.arange(NROW, dtype=np.int64) * 4 + R
        perm = np.concatenate(
            [np.arange(256 * H, 256 * H + 256), np.arange(256 * (1 - H), 256 * (2 - H))]
        )
        sel = ((a & 3) == R) & ((n >> 8) == H)
        asel, psel, nsel = a[sel], p[sel], n[sel]
        q = asel >> 2
        order = np.argsort(q, kind="stable")
        qs = q[order]
        counts = np.bincount(qs, minlength=NROW)
        if counts.max() > LCOL:
            raise ValueError("bucket overflow")
        starts = np.zeros(NROW, dtype=np.int64)
        starts[1:] = np.cumsum(counts)[:-1]
        slot = np.arange(len(qs)) - starts[qs]
        nbuk = np.full((NROW, LCOL), -1, dtype=np.int16)
        nbuk[qs, slot] = (nsel[order] & 255).astype(np.int16)

        # palettes: distinct positives per row; local col of raw id v:
        # (v & 255) + 256 * (v >> 8 != H)
        sidx = np.full((NROW, N), -1, dtype=np.int16)
        palidx1 = np.full((NROW, s_pad), -1.0, dtype=np.float32)  # -1 matches nothing
        rankof = {}
        ar = a[(a & 3) == R]
        pr = p[(a & 3) == R]
        rr_ = ar >> 2
        for qq in range(NROW):
            vals = np.unique(pr[rr_ == qq])
            if len(vals) > s_pad:
                raise ValueError("palette overflow")
            if len(vals) == 0:
                continue
            lcols = (vals & 255) + 256 * ((vals >> 8) != H)
            sidx[qq, lcols] = np.arange(len(vals), dtype=np.int16)
            palidx1[qq, : len(vals)] = vals + 1.0
            for s_, v in enumerate(vals):
                rankof[(qq, v)] = s_ + 1
        # rank+1 of each triplet's positive within its row palette
        pbuk = np.zeros((NROW, LCOL), dtype=np.float32)
        pbuk[qs, slot] = np.array(
            [rankof[(qqv, pv)] for qqv, pv in zip(qs, psel[order])], dtype=np.float32
        )
        pbuk = pbuk.astype(ml_bf16)
        ranks1 = np.broadcast_to(
            np.arange(1, s_pad + 1, dtype=np.float32), (NROW, s_pad)
        ).astype(ml_bf16)
        in_maps.append(
            {
                "stp": np.ascontiguousarray(samples[perm].T),
                "srt": np.ascontiguousarray(samples[rows].T),
                "sidx16": sidx,
                "palidx1f": palidx1,
                "nbuk16": nbuk,
                "pbuk16": pbuk,
                "ranks1": ranks1,
            }
        )
    return in_maps


def kernel(samples, targets, anchor_idx, pos_idx, neg_idx, _want_trace=False):
    samples = np.asarray(samples, dtype=np.float32)
    targets = np.asarray(targets).astype(np.int32)
    a = np.asarray(anchor_idx).astype(np.int64)
    p = np.asarray(pos_idx).astype(np.int64)
    n = np.asarray(neg_idx).astype(np.int64)
    T = a.shape[0]
    assert samples.shape == (N, D)

    ok = (
        np.all((a >= 0) & (a < N) & (p >= 0) & (p < N) & (n >= 0) & (n < N))
        and len(np.unique(a * N + n)) == T
    )
    if not ok:
        raise NotImplementedError("inputs violate mined-triplet structure")

    ap_pairs = np.unique(a * N + p)
    npal = np.bincount(ap_pairs // N, minlength=N)
    s_max = int(npal.max())
    s_pad = max(2, s_max + (s_max & 1))
    if s_pad > 32:
        raise NotImplementedError("palette too large for this kernel")

    nc = _get_program(s_pad)
    in_maps = _shard_inputs(samples, targets, a, p, n, s_pad)
    res = run_bass_kernel_spmd(nc, in_maps, list(range(NCORES)), trace=_want_trace)
    total = sum(float(res.results[c]["out"][0, 0]) for c in range(NCORES))
    loss = np.float32(total / T)
    if _want_trace:
        return loss, res
    return loss


# revision 15
# speedup vs baseline: 2.1088x; 1.0846x over previous
"""Trainium2 Bass kernel for BatchWiseTripletDistanceLoss.

Math: loss = mean_t relu(cos_d(s[a_t], s[p_t]) - cos_d(s[a_t], s[n_t]) + margin)
with cos_d(x, y) = 1 - <x,y>/max(|x||y|, eps).

Cosine distances depend only on (row, row) pairs of the 512x256 sample
matrix, so the kernel computes the 512x512 cosine-SIMILARITY matrix
sim = R S S^T R (R = diag(1/|s_i|)) on-device via TensorE and evaluates
    relu(sim[a,p] - sim[a,n] + margin)          ("1-" cancels in the diff)
on a dense [row, col] grid: each triplet is scattered to grid cell
(a_t, n_t) carrying p_t+1 (gpsimd local_scatter = true per-partition
scatter).  The distinct positives of each row form a small palette
(~13 entries here); palette VALUES are extracted from the sim row by a
second local_scatter, and a short loop over palette slots evaluates
masked relu terms, so no per-triplet gather is ever needed.

Sharding: 8 cores split the grid into (row mod 4) x (column half)
quadrants of [128, 256].  The host only transposes/permutes/buckets/pads
the given arrays (layout + palette metadata, no float math) and sums the
8 partial scalars at the end.
"""
import sys

sys.path.insert(0, "/opt/trn_rl_repo")

from contextlib import ExitStack

import numpy as np
import ml_dtypes

ml_bf16 = ml_dtypes.bfloat16

import concourse.bacc as bacc
import concourse.bass as bass
import concourse.tile as tile
from concourse import mybir
from concourse.bass_utils import run_bass_kernel_spmd

DT = mybir.dt
OP = mybir.AluOpType
ACTF = mybir.ActivationFunctionType

N = 512
D = 256
MARGIN = 0.15
NCORES = 8
LCOL = 256  # columns per core (half)
NROW = 128  # rows per core (stride-4 residue class)
NCHUNK = 4  # main-loop pipeline chunks


def _build_program(s_pad: int):
    """Build + compile the SPMD program (identical for all 8 cores)."""
    nc = bacc.Bacc(
        "TRN2", target_bir_lowering=False, debug=False, num_devices=NCORES
    )
    f32, i32, i16, f16 = DT.float32, DT.int32, DT.int16, DT.float16

    d_stp = nc.dram_tensor("stp", [D, N], f32, kind="ExternalInput").ap()
    d_srt = nc.dram_tensor("srt", [D, NROW], f32, kind="ExternalInput").ap()
    d_sidx = nc.dram_tensor("sidx16", [NROW, N], i16, kind="ExternalInput").ap()
    d_pal = nc.dram_tensor("palidx1f", [NROW, s_pad], f32, kind="ExternalInput").ap()
    d_nbuk = nc.dram_tensor("nbuk16", [NROW, LCOL], i16, kind="ExternalInput").ap()
    d_pbuk = nc.dram_tensor("pbuk16", [NROW, LCOL], DT.bfloat16, kind="ExternalInput").ap()
    d_ranks = nc.dram_tensor("ranks1", [NROW, s_pad], DT.bfloat16, kind="ExternalInput").ap()
    d_out = nc.dram_tensor("out", [NROW, NCHUNK], f32, kind="ExternalOutput").ap()

    with tile.TileContext(nc) as tc, ExitStack() as ctx:
        cpool = ctx.enter_context(tc.tile_pool(name="const", bufs=1))
        wpool = ctx.enter_context(tc.tile_pool(name="work", bufs=2))
        mpool = ctx.enter_context(tc.tile_pool(name="mainloop", bufs=4))
        ppool = ctx.enter_context(tc.tile_pool(name="psum", bufs=2, space="PSUM"))
        pfin = ctx.enter_context(tc.tile_pool(name="psumfin", bufs=1, space="PSUM"))
        pbig = ctx.enter_context(tc.tile_pool(name="psumbig", bufs=1, space="PSUM"))

        # ---- load inputs -------------------------------------------------
        st = []
        for k in range(2):
            t = cpool.tile([128, N], f32, tag=f"st{k}", name=f"st{k}")
            nc.sync.dma_start(t[:], d_stp[128 * k : 128 * (k + 1), :])
            st.append(t)
        sr = []
        for k in range(2):
            t = cpool.tile([128, NROW], f32, tag=f"sr{k}", name=f"sr{k}")
            nc.sync.dma_start(t[:], d_srt[128 * k : 128 * (k + 1), :])
            sr.append(t)
        nbuk16 = cpool.tile([NROW, LCOL], i16)
        nc.scalar.dma_start(nbuk16[:], d_nbuk)
        pbuk16 = cpool.tile([NROW, LCOL], DT.bfloat16)
        nc.scalar.dma_start(pbuk16[:], d_pbuk)
        sidx16 = cpool.tile([NROW, N], i16)
        nc.gpsimd.dma_start(sidx16[:], d_sidx)
        palidx1f = cpool.tile([NROW, s_pad], f32)
        nc.gpsimd.dma_start(palidx1f[:], d_pal)
        ranks1 = cpool.tile([NROW, s_pad], DT.bfloat16)
        nc.scalar.dma_start(ranks1[:], d_ranks)

        ones_col = cpool.tile([128, 1], f32)
        nc.vector.memset(ones_col[:], 1.0)
        ones_row1 = cpool.tile([1, 128], f32)
        nc.vector.memset(ones_row1[:], 1.0)

        # ---- preload ACT sqrt table during the DMA phase ----------------
        dumin = cpool.tile([1, 1], f32)
        nc.vector.memset(dumin[:], 4.0)
        dumout = cpool.tile([1, 1], f32)
        nc.scalar.sqrt(dumout[:], dumin[:])

        # ---- pidxg scatter (independent of samples) ---------------------
        pidxg = wpool.tile([NROW, LCOL], DT.bfloat16, tag="pidxg")
        nc.gpsimd.local_scatter(
            pidxg[:], pbuk16[:], nbuk16[:],
            channels=128, num_elems=LCOL, num_idxs=LCOL,
        )

        # ---- raw similarity matmul --------------------------------------
        simp = pbig.tile([128, N], f32, tag="simp")
        for k in range(2):
            nc.tensor.matmul(simp[:], sr[k][:], st[k][:], start=(k == 0), stop=(k == 1))

        # ---- norms ------------------------------------------------------
        sq = wpool.tile([128, N], f32, tag="sq")
        n2p = pbig.tile([1, N], f32, tag="n2row")
        for k in range(2):
            nc.scalar.square(sq[:], st[k][:])
            nc.tensor.matmul(n2p[:], ones_col[:], sq[:], start=(k == 0), stop=(k == 1))
        sqr = wpool.tile([128, NROW], f32, tag="sqr")
        n2rp = ppool.tile([128, 1], f32, tag="n2rp")
        for k in range(2):
            nc.scalar.square(sqr[:], sr[k][:])
            nc.tensor.matmul(n2rp[:], sqr[:], ones_col[:], start=(k == 0), stop=(k == 1))

        nrow = wpool.tile([1, N], f32, tag="nrow")
        nc.scalar.sqrt(nrow[:], n2p[:])
        nrr = wpool.tile([128, 1], f32, tag="nrr")
        nc.scalar.sqrt(nrr[:], n2rp[:])
        # preload relu table right after the sqrts (hidden off critical path)
        durelu = cpool.tile([1, 1], f32)
        nc.scalar.activation(durelu[:], dumout[:], ACTF.Relu)

        rrow = wpool.tile([1, N], f32, tag="rrow")
        rscr = wpool.tile([1, N], f32, tag="rscr")
        nc.vector.reciprocal_approx_accurate(rrow[:], nrow[:], rscr[:])
        rr = cpool.tile([128, 1], f32)
        rscr2 = wpool.tile([128, 1], f32, tag="rscr2")
        nc.vector.reciprocal_approx_accurate(rr[:], nrr[:], rscr2[:])

        # ---- combine: simrow = (raw . rr) x RB --------------------------
        rbp = pbig.tile([128, N], f32, tag="rb")
        nc.tensor.matmul(rbp[:], ones_row1[:], rrow[:], start=True, stop=True)
        t0 = wpool.tile([128, N], f32, tag="t0")
        nc.scalar.activation(t0[:], simp[:], ACTF.Copy, scale=rr[:])
        simrow = cpool.tile([128, N], f32)
        nc.vector.tensor_tensor(simrow[:], t0[:], rbp[:], OP.mult)
        sim16 = cpool.tile([128, N], f16)
        nc.scalar.copy(sim16[:], simrow[:])

        # ---- palette values + margin bias -------------------------------
        palv16 = wpool.tile([128, s_pad], f16, tag="palv16")
        nc.gpsimd.local_scatter(
            palv16[:], sim16[:], sidx16[:],
            channels=128, num_elems=s_pad, num_idxs=N,
        )
        palvf = wpool.tile([128, s_pad], f32, tag="palvf")
        nc.scalar.copy(palvf[:], palv16[:])
        mb = wpool.tile([128, s_pad], f32, tag="mb")
        nc.vector.tensor_scalar(mb[:], palvf[:], -1.0, MARGIN, OP.mult, OP.add)

        # ---- main palette loop (batched, chunked for pipelining) --------
        msall = wpool.tile([128, s_pad * LCOL], DT.bfloat16, tag="msall")
        msall_v = msall[:].rearrange("p (s j) -> p s j", s=s_pad)
        nc.vector.tensor_tensor(
            msall_v,
            pidxg[:].unsqueeze(1).to_broadcast((NROW, s_pad, LCOL)),
            ranks1[:].unsqueeze(2).to_broadcast((NROW, s_pad, LCOL)),
            OP.is_equal,
        )
        bounds = [(s_pad * c) // NCHUNK for c in range(NCHUNK + 1)]
        accs = wpool.tile([128, NCHUNK], f32, tag="accs")
        for c in range(NCHUNK):
            lo, hi = bounds[c], bounds[c + 1]
            w = (hi - lo) * LCOL
            t1c = mpool.tile([128, w], f32, tag="t1c", name=f"t1c{c}")
            nc.vector.tensor_tensor(
                t1c[:].rearrange("p (s j) -> p s j", s=hi - lo),
                simrow[:, 0:LCOL].unsqueeze(1).to_broadcast((NROW, hi - lo, LCOL)),
                mb[:, lo:hi].unsqueeze(2).to_broadcast((NROW, hi - lo, LCOL)),
                OP.add,
            )
            mkc = mpool.tile([128, w], f32, tag="mkc", name=f"mkc{c}")
            nc.vector.tensor_tensor(
                mkc[:], msall[:, lo * LCOL : hi * LCOL], t1c[:], OP.mult
            )
            rlc = mpool.tile([128, w], f32, tag="rlc", name=f"rlc{c}")
            nc.scalar.activation(
                rlc[:], mkc[:], ACTF.Relu, accum_out=accs[:, c : c + 1]
            )
        nc.sync.dma_start(d_out, accs[:])

    nc.compile()
    return nc


_PROGRAM_CACHE = {}


def _get_program(s_pad):
    if s_pad not in _PROGRAM_CACHE:
        _PROGRAM_CACHE[s_pad] = _build_program(s_pad)
    return _PROGRAM_CACHE[s_pad]


def _shard_inputs(samples, targets, a, p, n, s_pad):
    """Per-core layout: transpose/permute samples, bucket triplets, build
    palette metadata (distinct positives per row)."""
    in_maps = []
    for core in range(NCORES):
        R, H = core >> 1, core & 1
        rows = np.arange(NROW, dtype=np.int64) * 4 + R
        perm = np.concatenate(
            [np.arange(256 * H, 256 * H + 256), np.arange(256 * (1 - H), 256 * (2 - H))]
        )
        sel = ((a & 3) == R) & ((n >> 8) == H)
        asel, psel, nsel = a[sel], p[sel], n[sel]
        q = asel >> 2
        order = np.argsort(q, kind="stable")
        qs = q[order]
        counts = np.bincount(qs, minlength=NROW)
        if counts.max() > LCOL:
            raise ValueError("bucket overflow")
        starts = np.zeros(NROW, dtype=np.int64)
        starts[1:] = np.cumsum(counts)[:-1]
        slot = np.arange(len(qs)) - starts[qs]
        nbuk = np.full((NROW, LCOL), -1, dtype=np.int16)
        nbuk[qs, slot] = (nsel[order] & 255).astype(np.int16)

        # palettes: distinct positives per row; local col of raw id v:
        # (v & 255) + 256 * (v >> 8 != H)
        sidx = np.full((NROW, N), -1, dtype=np.int16)
        palidx1 = np.full((NROW, s_pad), -1.0, dtype=np.float32)  # -1 matches nothing
        rankof = {}
        ar = a[(a & 3) == R]
        pr = p[(a & 3) == R]
        rr_ = ar >> 2
        for qq in range(NROW):
            vals = np.unique(pr[rr_ == qq])
            if len(vals) > s_pad:
                raise ValueError("palette overflow")
            if len(vals) == 0:
                continue
            lcols = (vals & 255) + 256 * ((vals >> 8) != H)
            sidx[qq, lcols] = np.arange(len(vals), dtype=np.int16)
            palidx1[qq, : len(vals)] = vals + 1.0
            for s_, v in enumerate(vals):
                rankof[(qq, v)] = s_ + 1
        # rank+1 of each triplet's positive within its row palette
        pbuk = np.zeros((NROW, LCOL), dtype=np.float32)
        pbuk[qs, slot] = np.array(
            [rankof[(qqv, pv)] for qqv, pv in zip(qs, psel[order])], dtype=np.float32
        )
        pbuk = pbuk.astype(ml_bf16)
        ranks1 = np.broadcast_to(
            np.arange(1, s_pad + 1, dtype=np.float32), (NROW, s_pad)
        ).astype(ml_bf16)
        in_maps.append(
            {
                "stp": np.ascontiguousarray(samples[perm].T),
                "srt": np.ascontiguousarray(samples[rows].T),
                "sidx16": sidx,
                "palidx1f": palidx1,
                "nbuk16": nbuk,
                "pbuk16": pbuk,
                "ranks1": ranks1,
            }
        )
    return in_maps


def kernel(samples, targets, anchor_idx, pos_idx, neg_idx, _want_trace=False):
    samples = np.asarray(samples, dtype=np.float32)
    targets = np.asarray(targets).astype(np.int32)
    a = np.asarray(anchor_idx).astype(np.int64)
    p = np.asarray(pos_idx).astype(np.int64)
    n = np.asarray(neg_idx).astype(np.int64)
    T = a.shape[0]
    assert samples.shape == (N, D)

    ok = (
        np.all((a >= 0) & (a < N) & (p >= 0) & (p < N) & (n >= 0) & (n < N))
        and len(np.unique(a * N + n)) == T
    )
    if not ok:
        raise NotImplementedError("inputs violate mined-triplet structure")

    ap_pairs = np.unique(a * N + p)
    npal = np.bincount(ap_pairs // N, minlength=N)
    s_max = int(npal.max())
    s_pad = max(2, s_max + (s_max & 1))
    if s_pad > 32:
        raise NotImplementedError("palette too large for this kernel")

    nc = _get_program(s_pad)
    in_maps = _shard_inputs(samples, targets, a, p, n, s_pad)
    res = run_bass_kernel_spmd(nc, in_maps, list(range(NCORES)), trace=_want_trace)
    total = sum(float(res.results[c]["out"].astype(np.float64).sum()) for c in range(NCORES))
    loss = np.float32(total / T)
    if _want_trace:
        return loss, res
    return loss
